# revision 8
# baseline (speedup 1.0000x reference)
"""DeepFM (embedding_lookup) Trainium2 kernel.

Strategy: data-parallel over the batch. Each of the 8 NeuronCores handles
B/8 = 2048 samples with a replicated fused embedding table [V, 17]
(cols 0..15 = fm_emb, col 16 = fm_w). Per 128-sample tile the kernel:
  1. indirect-DMA gathers the 126 rows/sample (26 onehot + 2x50 multihot)
     into SBUF [128, 126*17],
  2. VectorE: mask-weighted bag means, field sums / sum-of-squares for the
     FM 2nd order term, first-order w sum,
  3. TensorE: transposes the 461-wide MLP input and runs the 2-layer MLP
     (biases folded in via a ones column / ones row),
  4. final 29-dim dot + sigmoid, DMA out.
"""

import numpy as np

import concourse.bass as bass
import concourse.tile as tile
from concourse import bacc, mybir
from concourse.bass import IndirectOffsetOnAxis
from concourse.bass_utils import run_bass_kernel_spmd
from concourse.masks import make_identity
from concourse.tile import TileContext

F32 = mybir.dt.float32
I32 = mybir.dt.int32
AF = mybir.ActivationFunctionType
OP = mybir.AluOpType

B, E, V, L = 16384, 16, 1_000_000, 50
DENSE, ONEHOT, MULTIHOT = 13, 26, 2
NFIELD = ONEHOT + MULTIHOT * L          # 126 gathered rows per sample
ROW = E + 1                             # fused table row: 16 emb + 1 w
NCORES = 8
BPC = B // NCORES                       # 2048 samples per core
TILE = 128
NT = BPC // TILE                        # 16 tiles per core
NCAT = ONEHOT + MULTIHOT                # 28 fields in cat_emb
CATW = NCAT * E                         # 448
NN_IN = CATW + DENSE                    # 461
XW = 512                                # padded MLP input width
U0, U1 = 64, 12
CONC = 1 + E + U1                       # 29

_CACHED = None
NQUEUES = 1
# host premultiplies indices by ROW so the SWDGE ucode computes addresses
# with coef=1 (saves a per-descriptor multiply on the Q7)
PREMUL = True


def _indirect_gather_q(nc, out, in_, offset_ap, coef, queue_num):
    """indirect_dma_start pinned to a specific SWDGE queue so descriptor
    generation spreads across GPSIMD Q7 core pairs."""
    out_ap = nc.gpsimd.lower_ap_dma(out, for_indirect_dma=True)
    in_ap = nc.gpsimd.lower_ap_dma(in_, for_indirect_dma=True)
    assert len(in_ap) == 1 and len(out_ap) == 1
    off = nc.gpsimd.lower_ap_dma(offset_ap)
    assert len(off) == 1
    in_ap.append(off[0])
    in_ap[0].dynamic_ap_info = mybir.DynamicAccessPatternInfo(
        c=0, actual_ap=out.ap, indirect_dim_max_index=in_.shape[0],
        offset_expr=[mybir.DynamicAccessPatternOffsetExpr(
            coef=coef,
            aff_expr=mybir.DynamicAccessPatternOffsetExprAffExpr(
                kind="IndirectArgId", arg_id=1))])
    return nc.gpsimd.add_instruction(
        mybir.InstDMACopy(
            name=nc.get_next_instruction_name(),
            queue=f"qPoolDynamic{queue_num or ''}",
            mode="Copy",
            ins=in_ap, outs=out_ap,
            oob_is_err=False,
            cce_op=mybir.AluOpType.bypass,
        ))


def _build(bpc=BPC, v=V, num_devices=NCORES, stage="full"):
    import os
    stage = os.environ.get("KSTAGE", stage)
    nt = bpc // TILE
    nc = bacc.Bacc("TRN2", target_bir_lowering=False, debug=False,
                   num_devices=num_devices, num_swdge_queues=NQUEUES)

    table_d = nc.dram_tensor("table", [v, ROW], F32, kind="ExternalInput")
    idx_d = nc.dram_tensor("idx", [bpc, NFIELD], I32, kind="ExternalInput")
    mask_d = nc.dram_tensor("mask", [bpc, MULTIHOT * L], F32,
                            kind="ExternalInput")
    dense_d = nc.dram_tensor("dense", [bpc, DENSE], F32, kind="ExternalInput")
    w0_d = nc.dram_tensor("w0", [XW, U0], F32, kind="ExternalInput")
    w1_d = nc.dram_tensor("w1b", [U0 + 1, U1], F32, kind="ExternalInput")
    cw_d = nc.dram_tensor("cw", [CONC], F32, kind="ExternalInput")
    cb_d = nc.dram_tensor("cb", [1], F32, kind="ExternalInput")
    out_d = nc.dram_tensor("out", [bpc, 1], F32, kind="ExternalOutput")

    with TileContext(nc) as tc:
        with (
            tc.tile_pool(name="singles", bufs=1) as singles,
            tc.tile_pool(name="gather", bufs=3) as gpool,
            tc.tile_pool(name="io", bufs=3) as iopool,
            tc.tile_pool(name="x", bufs=3) as xpool,
            tc.tile_pool(name="xt", bufs=2) as xtpool,
            tc.tile_pool(name="sq", bufs=2) as sqpool,
            tc.tile_pool(name="small", bufs=4) as spool,
            tc.tile_pool(name="res", bufs=3) as rpool,
            tc.tile_pool(name="psA", bufs=2, space="PSUM") as psA,
            tc.tile_pool(name="psB", bufs=2, space="PSUM") as psB,
            tc.tile_pool(name="psC", bufs=2, space="PSUM") as psC,
            tc.tile_pool(name="psD", bufs=2, space="PSUM") as psD,
        ):
            ident = singles.tile([128, 128], F32)
            make_identity(nc, ident[:])
            w0_s = singles.tile([128, 4 * U0], F32)
            for k in range(4):
                nc.sync.dma_start(out=w0_s[:, k * U0:(k + 1) * U0],
                                  in_=w0_d.ap()[k * 128:(k + 1) * 128, :])
            w1_s = singles.tile([U0 + 1, U1], F32)
            nc.sync.dma_start(out=w1_s[:], in_=w1_d.ap())
            cw_s = singles.tile([128, CONC], F32)
            nc.sync.dma_start(
                out=cw_s[:],
                in_=bass.AP(tensor=cw_d, offset=0, ap=[[0, 128], [1, CONC]]))
            cb_s = singles.tile([128, 1], F32)
            nc.sync.dma_start(
                out=cb_s[:],
                in_=bass.AP(tensor=cb_d, offset=0, ap=[[0, 128], [1, 1]]))

            for t in range(nt):
                r0, r1 = t * TILE, (t + 1) * TILE

                idx_t = iopool.tile([TILE, NFIELD], I32, tag="idx")
                nc.sync.dma_start(out=idx_t[:], in_=idx_d.ap()[r0:r1, :])
                mask_t = iopool.tile([TILE, MULTIHOT * L], F32, tag="mask")
                nc.sync.dma_start(out=mask_t[:], in_=mask_d.ap()[r0:r1, :])

                # --- gather 126 fused rows per sample ---
                # HW indirect DMA consumes one offset per partition; each
                # instruction gathers 128 rows (one per sample) for one field.
                G = gpool.tile([TILE, NFIELD * ROW], F32)
                G3 = G[:].rearrange("p (f r) -> p f r", r=ROW)
                for f in range(NFIELD):
                    _indirect_gather_q(
                        nc, G3[:, f, :], table_d.ap(),
                        idx_t[:, f:f + 1], 1 if PREMUL else ROW,
                        f % NQUEUES)

                # --- X = [cat_emb(448) | dense(13) | 1.0 | zeros] ---
                X = xpool.tile([TILE, XW], F32)
                nc.sync.dma_start(out=X[:, CATW:NN_IN],
                                  in_=dense_d.ap()[r0:r1, :])
                nc.vector.memset(X[:, NN_IN:NN_IN + 1], 1.0)
                nc.vector.memset(X[:, NN_IN + 1:XW], 0.0)

                if stage == "gather":
                    res = rpool.tile([TILE, 1], F32, tag="res")
                    nc.vector.tensor_copy(out=res[:], in_=G[:, 0:1])
                    nc.sync.dma_start(out=out_d.ap()[r0:r1, :], in_=res[:])
                    continue

                # multihot: multiply mask into gathered rows in place
                mh = G3[:, ONEHOT:NFIELD, 0:E]               # [128,100,16]
                mb = mask_t[:].unsqueeze(2).to_broadcast(
                    [TILE, MULTIHOT * L, E])
                nc.vector.tensor_mul(out=mh, in0=mh, in1=mb)
                # bag sums -> X[:, 416:448]  (view [128, bag, j] <- sum over l)
                mh_sum_view = X[:, CATW - MULTIHOT * E:CATW].rearrange(
                    "p (b j) -> p b j", b=MULTIHOT)
                nc.vector.reduce_sum(
                    out=mh_sum_view,
                    in_=_mh_view(G, ONEHOT),
                    axis=mybir.AxisListType.X)
                # denominators
                D = spool.tile([TILE, MULTIHOT], F32, tag="den")
                nc.vector.reduce_sum(
                    out=D[:],
                    in_=mask_t[:].rearrange("p (b l) -> p b l", b=MULTIHOT),
                    axis=mybir.AxisListType.X)
                nc.vector.tensor_scalar_max(out=D[:], in0=D[:], scalar1=1.0)
                R = spool.tile([TILE, MULTIHOT], F32, tag="rec")
                nc.vector.reciprocal(out=R[:], in_=D[:])
                nc.vector.tensor_mul(
                    out=mh_sum_view, in0=mh_sum_view,
                    in1=R[:].unsqueeze(2).to_broadcast([TILE, MULTIHOT, E]))

                if stage == "mh":
                    res = rpool.tile([TILE, 1], F32, tag="res")
                    nc.vector.tensor_copy(out=res[:], in_=X[:, 416:417])
                    nc.sync.dma_start(out=out_d.ap()[r0:r1, :], in_=res[:])
                    continue

                # onehot embeddings -> X[:, 0:416] (ACT engine copy)
                nc.scalar.copy(
                    out=X[:, 0:ONEHOT * E].rearrange("p (f j) -> p f j", j=E),
                    in_=G3[:, 0:ONEHOT, 0:E])

                # --- FM terms ---
                C = rpool.tile([TILE, CONC], F32, tag="C")
                # fm_1st = sum of w col (col 16 of each onehot row)
                nc.vector.reduce_sum(
                    out=C[:, 0:1],
                    in_=G3[:, 0:ONEHOT, E:ROW].squeeze(2),
                    axis=mybir.AxisListType.X)
                # s = sum over 28 fields
                s_t = spool.tile([TILE, E], F32, tag="s")
                nc.vector.reduce_sum(
                    out=s_t[:],
                    in_=X[:, 0:CATW].rearrange("p (f j) -> p j f", f=NCAT),
                    axis=mybir.AxisListType.X)
                # sum of squares over 28 fields
                SQ = sqpool.tile([TILE, CATW], F32)
                nc.scalar.square(out=SQ[:], in_=X[:, 0:CATW])
                ss_t = spool.tile([TILE, E], F32, tag="ss")
                nc.vector.reduce_sum(
                    out=ss_t[:],
                    in_=SQ[:].rearrange("p (f j) -> p j f", f=NCAT),
                    axis=mybir.AxisListType.X)
                # fm2 = 0.5*(s*s - ss) -> C[:, 1:17]
                t16 = spool.tile([TILE, E], F32, tag="t16")
                nc.vector.tensor_mul(out=t16[:], in0=s_t[:], in1=s_t[:])
                nc.vector.tensor_sub(out=t16[:], in0=t16[:], in1=ss_t[:])
                nc.scalar.mul(out=C[:, 1:1 + E], in_=t16[:], mul=0.5)

                if stage == "fm":
                    res = rpool.tile([TILE, 1], F32, tag="res")
                    nc.vector.tensor_copy(out=res[:], in_=C[:, 1:2])
                    nc.sync.dma_start(out=out_d.ap()[r0:r1, :], in_=res[:])
                    continue

                # --- MLP ---
                XTp = psA.tile([128, XW], F32)
                for k in range(4):
                    nc.tensor.transpose(out=XTp[:, k * 128:(k + 1) * 128],
                                        in_=X[:, k * 128:(k + 1) * 128],
                                        identity=ident[:])
                XTs = xtpool.tile([128, XW], F32)
                nc.scalar.copy(out=XTs[:], in_=XTp[:])
                h0p = psB.tile([128, U0], F32)
                for k in range(4):
                    nc.tensor.matmul(out=h0p[:],
                                     lhsT=XTs[:, k * 128:(k + 1) * 128],
                                     rhs=w0_s[:, k * U0:(k + 1) * U0],
                                     start=(k == 0), stop=(k == 3))
                h0s = spool.tile([128, U0], F32, tag="h0")
                nc.scalar.activation(out=h0s[:], in_=h0p[:], func=AF.Relu)
                h0Tp = psC.tile([U0, 128], F32)
                nc.tensor.transpose(out=h0Tp[:], in_=h0s[:], identity=ident[:])
                h0Ts = xtpool.tile([U0 + 1, 128], F32, tag="h0T")
                nc.vector.tensor_copy(out=h0Ts[0:U0, :], in_=h0Tp[:])
                nc.vector.memset(h0Ts[U0:U0 + 1, :], 1.0)
                h1p = psD.tile([128, U1], F32)
                nc.tensor.matmul(out=h1p[:], lhsT=h0Ts[:], rhs=w1_s[:],
                                 start=True, stop=True)
                nc.scalar.activation(out=C[:, 1 + E:CONC], in_=h1p[:],
                                     func=AF.Relu)

                if stage == "mlp":
                    res = rpool.tile([TILE, 1], F32, tag="res")
                    nc.vector.tensor_copy(out=res[:], in_=C[:, 17:18])
                    nc.sync.dma_start(out=out_d.ap()[r0:r1, :], in_=res[:])
                    continue

                # --- final dot + sigmoid ---
                scr = rpool.tile([TILE, CONC], F32, tag="scr")
                acc = rpool.tile([TILE, 1], F32, tag="acc")
                nc.vector.tensor_mul(out=scr[:], in0=C[:], in1=cw_s[:])
                nc.vector.reduce_sum(out=acc[:], in_=scr[:],
                                     axis=mybir.AxisListType.X)
                nc.vector.tensor_scalar_add(out=acc[:], in0=acc[:],
                                            scalar1=cb_s[:, 0:1])
                res = rpool.tile([TILE, 1], F32, tag="res")
                nc.scalar.activation(out=res[:], in_=acc[:], func=AF.Sigmoid)
                nc.sync.dma_start(out=out_d.ap()[r0:r1, :], in_=res[:])

    nc.compile()
    return nc


def _mh_view(G, start_field):
    """[128, bag, j, l] view of the multihot region of G for the bag-sum
    reduce: bag stride L*ROW, j stride 1, l stride ROW."""
    base = G[:, start_field * ROW:]
    return bass.AP(
        tensor=base.tensor, offset=base.offset,
        ap=[base.ap[0], [L * ROW, MULTIHOT], [1, E], [ROW, L]])


def _get_nc():
    global _CACHED
    if _CACHED is None:
        _CACHED = _build()
    return _CACHED


def kernel(dense, onehot, multihot_ids, multihot_mask, fm_w, fm_emb,
           nn_w0, nn_b0, nn_w1, nn_b1, concat_w, concat_b):
    dense = np.asarray(dense, np.float32)
    onehot = np.asarray(onehot, np.int32)
    multihot_ids = np.asarray(multihot_ids, np.int32)
    multihot_mask = np.asarray(multihot_mask, np.float32)
    fm_w = np.asarray(fm_w, np.float32)
    fm_emb = np.asarray(fm_emb, np.float32)

    table = np.ascontiguousarray(
        np.concatenate([fm_emb, fm_w], axis=1), np.float32)     # [V, 17]
    idx_all = np.concatenate(
        [onehot, multihot_ids.reshape(B, MULTIHOT * L)], axis=1).astype(
            np.int32)                                           # [B, 126]
    if PREMUL:
        idx_all = idx_all * ROW
    mask_all = multihot_mask.reshape(B, MULTIHOT * L)
    w0p = np.zeros((XW, U0), np.float32)
    w0p[:NN_IN] = np.asarray(nn_w0, np.float32)
    w0p[NN_IN] = np.asarray(nn_b0, np.float32)                  # ones-col bias
    w1b = np.concatenate(
        [np.asarray(nn_w1, np.float32),
         np.asarray(nn_b1, np.float32)[None, :]], axis=0)       # [65, 12]
    cw = np.ascontiguousarray(np.asarray(concat_w, np.float32)[:, 0])
    cb = np.asarray(concat_b, np.float32).reshape(1)

    nc = _get_nc()
    in_maps = []
    for c in range(NCORES):
        sl = slice(c * BPC, (c + 1) * BPC)
        in_maps.append({
            "table": table,
            "idx": np.ascontiguousarray(idx_all[sl]),
            "mask": np.ascontiguousarray(mask_all[sl]),
            "dense": np.ascontiguousarray(dense[sl]),
            "w0": w0p,
            "w1b": w1b,
            "cw": cw,
            "cb": cb,
        })
    global _LAST_IN_MAPS
    _LAST_IN_MAPS = in_maps
    r = run_bass_kernel_spmd(nc, in_maps, core_ids=list(range(NCORES)))
    return np.concatenate([m["out"] for m in r.results], axis=0)


_LAST_IN_MAPS = None



# revision 9
# speedup vs baseline: 1.0006x; 1.0006x over previous
"""DeepFM (embedding_lookup) Trainium2 kernel.

Strategy: data-parallel over the batch. Each of the 8 NeuronCores handles
B/8 = 2048 samples with a replicated fused embedding table [V, 17]
(cols 0..15 = fm_emb, col 16 = fm_w). Per 128-sample tile the kernel:
  1. indirect-DMA gathers the 126 rows/sample (26 onehot + 2x50 multihot)
     into SBUF [128, 126*17],
  2. VectorE: mask-weighted bag means, field sums / sum-of-squares for the
     FM 2nd order term, first-order w sum,
  3. TensorE: transposes the 461-wide MLP input and runs the 2-layer MLP
     (biases folded in via a ones column / ones row),
  4. final 29-dim dot + sigmoid, DMA out.
"""

import numpy as np

import concourse.bass as bass
import concourse.tile as tile
from concourse import bacc, mybir
from concourse.bass import IndirectOffsetOnAxis
from concourse.bass_utils import run_bass_kernel_spmd
from concourse.masks import make_identity
from concourse.tile import TileContext

F32 = mybir.dt.float32
I32 = mybir.dt.int32
AF = mybir.ActivationFunctionType
OP = mybir.AluOpType

B, E, V, L = 16384, 16, 1_000_000, 50
DENSE, ONEHOT, MULTIHOT = 13, 26, 2
NFIELD = ONEHOT + MULTIHOT * L          # 126 gathered rows per sample
ROW = E + 1                             # fused table row: 16 emb + 1 w
NCORES = 8
BPC = B // NCORES                       # 2048 samples per core
TILE = 128
NT = BPC // TILE                        # 16 tiles per core
NCAT = ONEHOT + MULTIHOT                # 28 fields in cat_emb
CATW = NCAT * E                         # 448
NN_IN = CATW + DENSE                    # 461
XW = 512                                # padded MLP input width
U0, U1 = 64, 12
CONC = 1 + E + U1                       # 29

_CACHED = None
NQUEUES = 4
# host premultiplies indices by ROW so the SWDGE ucode computes addresses
# with coef=1 (saves a per-descriptor multiply on the Q7)
PREMUL = True


def _indirect_gather_q(nc, out, in_, offset_ap, coef, queue_num):
    """indirect_dma_start pinned to a specific SWDGE queue so descriptor
    generation spreads across GPSIMD Q7 core pairs."""
    out_ap = nc.gpsimd.lower_ap_dma(out, for_indirect_dma=True)
    in_ap = nc.gpsimd.lower_ap_dma(in_, for_indirect_dma=True)
    assert len(in_ap) == 1 and len(out_ap) == 1
    off = nc.gpsimd.lower_ap_dma(offset_ap)
    assert len(off) == 1
    in_ap.append(off[0])
    in_ap[0].dynamic_ap_info = mybir.DynamicAccessPatternInfo(
        c=0, actual_ap=out.ap, indirect_dim_max_index=in_.shape[0],
        offset_expr=[mybir.DynamicAccessPatternOffsetExpr(
            coef=coef,
            aff_expr=mybir.DynamicAccessPatternOffsetExprAffExpr(
                kind="IndirectArgId", arg_id=1))])
    return nc.gpsimd.add_instruction(
        mybir.InstDMACopy(
            name=nc.get_next_instruction_name(),
            queue=f"qPoolDynamic{queue_num or ''}",
            mode="Copy",
            ins=in_ap, outs=out_ap,
            oob_is_err=True,
            cce_op=mybir.AluOpType.bypass,
        ))


def _build(bpc=BPC, v=V, num_devices=NCORES, stage="full"):
    import os
    stage = os.environ.get("KSTAGE", stage)
    nt = bpc // TILE
    nc = bacc.Bacc("TRN2", target_bir_lowering=False, debug=False,
                   num_devices=num_devices, num_swdge_queues=NQUEUES)

    table_d = nc.dram_tensor("table", [v, ROW], F32, kind="ExternalInput")
    idx_d = nc.dram_tensor("idx", [bpc, NFIELD], I32, kind="ExternalInput")
    mask_d = nc.dram_tensor("mask", [bpc, MULTIHOT * L], F32,
                            kind="ExternalInput")
    dense_d = nc.dram_tensor("dense", [bpc, DENSE], F32, kind="ExternalInput")
    w0_d = nc.dram_tensor("w0", [XW, U0], F32, kind="ExternalInput")
    w1_d = nc.dram_tensor("w1b", [U0 + 1, U1], F32, kind="ExternalInput")
    cw_d = nc.dram_tensor("cw", [CONC], F32, kind="ExternalInput")
    cb_d = nc.dram_tensor("cb", [1], F32, kind="ExternalInput")
    out_d = nc.dram_tensor("out", [bpc, 1], F32, kind="ExternalOutput")

    with TileContext(nc) as tc:
        with (
            tc.tile_pool(name="singles", bufs=1) as singles,
            tc.tile_pool(name="gather", bufs=3) as gpool,
            tc.tile_pool(name="io", bufs=3) as iopool,
            tc.tile_pool(name="x", bufs=3) as xpool,
            tc.tile_pool(name="xt", bufs=2) as xtpool,
            tc.tile_pool(name="sq", bufs=2) as sqpool,
            tc.tile_pool(name="small", bufs=4) as spool,
            tc.tile_pool(name="res", bufs=3) as rpool,
            tc.tile_pool(name="psA", bufs=2, space="PSUM") as psA,
            tc.tile_pool(name="psB", bufs=2, space="PSUM") as psB,
            tc.tile_pool(name="psC", bufs=2, space="PSUM") as psC,
            tc.tile_pool(name="psD", bufs=2, space="PSUM") as psD,
        ):
            ident = singles.tile([128, 128], F32)
            make_identity(nc, ident[:])
            w0_s = singles.tile([128, 4 * U0], F32)
            for k in range(4):
                nc.sync.dma_start(out=w0_s[:, k * U0:(k + 1) * U0],
                                  in_=w0_d.ap()[k * 128:(k + 1) * 128, :])
            w1_s = singles.tile([U0 + 1, U1], F32)
            nc.sync.dma_start(out=w1_s[:], in_=w1_d.ap())
            cw_s = singles.tile([128, CONC], F32)
            nc.sync.dma_start(
                out=cw_s[:],
                in_=bass.AP(tensor=cw_d, offset=0, ap=[[0, 128], [1, CONC]]))
            cb_s = singles.tile([128, 1], F32)
            nc.sync.dma_start(
                out=cb_s[:],
                in_=bass.AP(tensor=cb_d, offset=0, ap=[[0, 128], [1, 1]]))

            for t in range(nt):
                r0, r1 = t * TILE, (t + 1) * TILE

                idx_t = iopool.tile([TILE, NFIELD], I32, tag="idx")
                nc.sync.dma_start(out=idx_t[:], in_=idx_d.ap()[r0:r1, :])
                mask_t = iopool.tile([TILE, MULTIHOT * L], F32, tag="mask")
                nc.sync.dma_start(out=mask_t[:], in_=mask_d.ap()[r0:r1, :])

                # --- gather 126 fused rows per sample ---
                # HW indirect DMA consumes one offset per partition; each
                # instruction gathers 128 rows (one per sample) for one field.
                G = gpool.tile([TILE, NFIELD * ROW], F32)
                G3 = G[:].rearrange("p (f r) -> p f r", r=ROW)
                for f in range(NFIELD):
                    _indirect_gather_q(
                        nc, G3[:, f, :], table_d.ap(),
                        idx_t[:, f:f + 1], 1 if PREMUL else ROW,
                        f % NQUEUES)

                # --- X = [cat_emb(448) | dense(13) | 1.0 | zeros] ---
                X = xpool.tile([TILE, XW], F32)
                nc.sync.dma_start(out=X[:, CATW:NN_IN],
                                  in_=dense_d.ap()[r0:r1, :])
                nc.vector.memset(X[:, NN_IN:NN_IN + 1], 1.0)
                nc.vector.memset(X[:, NN_IN + 1:XW], 0.0)

                if stage == "gather":
                    res = rpool.tile([TILE, 1], F32, tag="res")
                    nc.vector.tensor_copy(out=res[:], in_=G[:, 0:1])
                    nc.sync.dma_start(out=out_d.ap()[r0:r1, :], in_=res[:])
                    continue

                # multihot: multiply mask into gathered rows in place
                mh = G3[:, ONEHOT:NFIELD, 0:E]               # [128,100,16]
                mb = mask_t[:].unsqueeze(2).to_broadcast(
                    [TILE, MULTIHOT * L, E])
                nc.vector.tensor_mul(out=mh, in0=mh, in1=mb)
                # bag sums -> X[:, 416:448]  (view [128, bag, j] <- sum over l)
                mh_sum_view = X[:, CATW - MULTIHOT * E:CATW].rearrange(
                    "p (b j) -> p b j", b=MULTIHOT)
                nc.vector.reduce_sum(
                    out=mh_sum_view,
                    in_=_mh_view(G, ONEHOT),
                    axis=mybir.AxisListType.X)
                # denominators
                D = spool.tile([TILE, MULTIHOT], F32, tag="den")
                nc.vector.reduce_sum(
                    out=D[:],
                    in_=mask_t[:].rearrange("p (b l) -> p b l", b=MULTIHOT),
                    axis=mybir.AxisListType.X)
                nc.vector.tensor_scalar_max(out=D[:], in0=D[:], scalar1=1.0)
                R = spool.tile([TILE, MULTIHOT], F32, tag="rec")
                nc.vector.reciprocal(out=R[:], in_=D[:])
                nc.vector.tensor_mul(
                    out=mh_sum_view, in0=mh_sum_view,
                    in1=R[:].unsqueeze(2).to_broadcast([TILE, MULTIHOT, E]))

                if stage == "mh":
                    res = rpool.tile([TILE, 1], F32, tag="res")
                    nc.vector.tensor_copy(out=res[:], in_=X[:, 416:417])
                    nc.sync.dma_start(out=out_d.ap()[r0:r1, :], in_=res[:])
                    continue

                # onehot embeddings -> X[:, 0:416] (ACT engine copy)
                nc.scalar.copy(
                    out=X[:, 0:ONEHOT * E].rearrange("p (f j) -> p f j", j=E),
                    in_=G3[:, 0:ONEHOT, 0:E])

                # --- FM terms ---
                C = rpool.tile([TILE, CONC], F32, tag="C")
                # fm_1st = sum of w col (col 16 of each onehot row)
                nc.vector.reduce_sum(
                    out=C[:, 0:1],
                    in_=G3[:, 0:ONEHOT, E:ROW].squeeze(2),
                    axis=mybir.AxisListType.X)
                # s = sum over 28 fields
                s_t = spool.tile([TILE, E], F32, tag="s")
                nc.vector.reduce_sum(
                    out=s_t[:],
                    in_=X[:, 0:CATW].rearrange("p (f j) -> p j f", f=NCAT),
                    axis=mybir.AxisListType.X)
                # sum of squares over 28 fields
                SQ = sqpool.tile([TILE, CATW], F32)
                nc.scalar.square(out=SQ[:], in_=X[:, 0:CATW])
                ss_t = spool.tile([TILE, E], F32, tag="ss")
                nc.vector.reduce_sum(
                    out=ss_t[:],
                    in_=SQ[:].rearrange("p (f j) -> p j f", f=NCAT),
                    axis=mybir.AxisListType.X)
                # fm2 = 0.5*(s*s - ss) -> C[:, 1:17]
                t16 = spool.tile([TILE, E], F32, tag="t16")
                nc.vector.tensor_mul(out=t16[:], in0=s_t[:], in1=s_t[:])
                nc.vector.tensor_sub(out=t16[:], in0=t16[:], in1=ss_t[:])
                nc.scalar.mul(out=C[:, 1:1 + E], in_=t16[:], mul=0.5)

                if stage == "fm":
                    res = rpool.tile([TILE, 1], F32, tag="res")
                    nc.vector.tensor_copy(out=res[:], in_=C[:, 1:2])
                    nc.sync.dma_start(out=out_d.ap()[r0:r1, :], in_=res[:])
                    continue

                # --- MLP ---
                XTp = psA.tile([128, XW], F32)
                for k in range(4):
                    nc.tensor.transpose(out=XTp[:, k * 128:(k + 1) * 128],
                                        in_=X[:, k * 128:(k + 1) * 128],
                                        identity=ident[:])
                XTs = xtpool.tile([128, XW], F32)
                nc.scalar.copy(out=XTs[:], in_=XTp[:])
                h0p = psB.tile([128, U0], F32)
                for k in range(4):
                    nc.tensor.matmul(out=h0p[:],
                                     lhsT=XTs[:, k * 128:(k + 1) * 128],
                                     rhs=w0_s[:, k * U0:(k + 1) * U0],
                                     start=(k == 0), stop=(k == 3))
                h0s = spool.tile([128, U0], F32, tag="h0")
                nc.scalar.activation(out=h0s[:], in_=h0p[:], func=AF.Relu)
                h0Tp = psC.tile([U0, 128], F32)
                nc.tensor.transpose(out=h0Tp[:], in_=h0s[:], identity=ident[:])
                h0Ts = xtpool.tile([U0 + 1, 128], F32, tag="h0T")
                nc.vector.tensor_copy(out=h0Ts[0:U0, :], in_=h0Tp[:])
                nc.vector.memset(h0Ts[U0:U0 + 1, :], 1.0)
                h1p = psD.tile([128, U1], F32)
                nc.tensor.matmul(out=h1p[:], lhsT=h0Ts[:], rhs=w1_s[:],
                                 start=True, stop=True)
                nc.scalar.activation(out=C[:, 1 + E:CONC], in_=h1p[:],
                                     func=AF.Relu)

                if stage == "mlp":
                    res = rpool.tile([TILE, 1], F32, tag="res")
                    nc.vector.tensor_copy(out=res[:], in_=C[:, 17:18])
                    nc.sync.dma_start(out=out_d.ap()[r0:r1, :], in_=res[:])
                    continue

                # --- final dot + sigmoid ---
                scr = rpool.tile([TILE, CONC], F32, tag="scr")
                acc = rpool.tile([TILE, 1], F32, tag="acc")
                nc.vector.tensor_mul(out=scr[:], in0=C[:], in1=cw_s[:])
                nc.vector.reduce_sum(out=acc[:], in_=scr[:],
                                     axis=mybir.AxisListType.X)
                nc.vector.tensor_scalar_add(out=acc[:], in0=acc[:],
                                            scalar1=cb_s[:, 0:1])
                res = rpool.tile([TILE, 1], F32, tag="res")
                nc.scalar.activation(out=res[:], in_=acc[:], func=AF.Sigmoid)
                nc.sync.dma_start(out=out_d.ap()[r0:r1, :], in_=res[:])

    nc.compile()
    return nc


def _mh_view(G, start_field):
    """[128, bag, j, l] view of the multihot region of G for the bag-sum
    reduce: bag stride L*ROW, j stride 1, l stride ROW."""
    base = G[:, start_field * ROW:]
    return bass.AP(
        tensor=base.tensor, offset=base.offset,
        ap=[base.ap[0], [L * ROW, MULTIHOT], [1, E], [ROW, L]])


def _get_nc():
    global _CACHED
    if _CACHED is None:
        _CACHED = _build()
    return _CACHED


def kernel(dense, onehot, multihot_ids, multihot_mask, fm_w, fm_emb,
           nn_w0, nn_b0, nn_w1, nn_b1, concat_w, concat_b):
    dense = np.asarray(dense, np.float32)
    onehot = np.asarray(onehot, np.int32)
    multihot_ids = np.asarray(multihot_ids, np.int32)
    multihot_mask = np.asarray(multihot_mask, np.float32)
    fm_w = np.asarray(fm_w, np.float32)
    fm_emb = np.asarray(fm_emb, np.float32)

    table = np.ascontiguousarray(
        np.concatenate([fm_emb, fm_w], axis=1), np.float32)     # [V, 17]
    idx_all = np.concatenate(
        [onehot, multihot_ids.reshape(B, MULTIHOT * L)], axis=1).astype(
            np.int32)                                           # [B, 126]
    if PREMUL:
        idx_all = idx_all * ROW
    mask_all = multihot_mask.reshape(B, MULTIHOT * L)
    w0p = np.zeros((XW, U0), np.float32)
    w0p[:NN_IN] = np.asarray(nn_w0, np.float32)
    w0p[NN_IN] = np.asarray(nn_b0, np.float32)                  # ones-col bias
    w1b = np.concatenate(
        [np.asarray(nn_w1, np.float32),
         np.asarray(nn_b1, np.float32)[None, :]], axis=0)       # [65, 12]
    cw = np.ascontiguousarray(np.asarray(concat_w, np.float32)[:, 0])
    cb = np.asarray(concat_b, np.float32).reshape(1)

    nc = _get_nc()
    in_maps = []
    for c in range(NCORES):
        sl = slice(c * BPC, (c + 1) * BPC)
        in_maps.append({
            "table": table,
            "idx": np.ascontiguousarray(idx_all[sl]),
            "mask": np.ascontiguousarray(mask_all[sl]),
            "dense": np.ascontiguousarray(dense[sl]),
            "w0": w0p,
            "w1b": w1b,
            "cw": cw,
            "cb": cb,
        })
    global _LAST_IN_MAPS
    _LAST_IN_MAPS = in_maps
    r = run_bass_kernel_spmd(nc, in_maps, core_ids=list(range(NCORES)))
    return np.concatenate([m["out"] for m in r.results], axis=0)


_LAST_IN_MAPS = None

